# revision 55
# baseline (speedup 1.0000x reference)
"""Trainium2 Bass kernel for LocalDualDirectedMessagePassingLayer.

Strategy (8 cores, dest-sharded):
  - Each core owns 1024 destination segments (8 blocks of 128 dests).
  - dest_seg is sorted, so each dest block's edges are contiguous; host pads
    each block's edge list to BLOCK_CAP = ST_B*512 and packs, per core,
    feature-major (transposed) dense bf16 operands:
      srcT [2,128,E_CAP]  = concat(node_memory,node_features)[source_ids].T
      efts [97,E_CAP]     = concat(edge_features[edge_ids], time_encoding, ones).T
    (the ones row folds b_msg into the msg-MLP matmul).
  - Device per 512-edge super-tile: read MLP out [j,512] via 2 K-tile matmuls
    (lhsT=W_read, rhs=srcT) + ACT relu(+b_read); per 128-edge sub-tile:
    msg MLP out [e,128] (lhsT=activations, rhs=W_msg k-tiles) + DVE
    relu*scale (scale=1/cnt folded per edge, 0 for padding); one-hot S from
    iota==ldest on gpsimd; aggregation matmul accumulates msg_mean^T [j,d]
    into PSUM across the block.
  - Per block: dst-side MLP chain (agg/upd/write) -> tanh -> writeT [128,1024].
  - Host: transpose writeT, scatter rows into a copy of node_memory.
All matmul operands bf16, PSUM accumulation fp32.
"""

import sys

sys.path.insert(0, "/opt/trn_rl_repo")

import math

import ml_dtypes
import numpy as np

import concourse.bass as bass
import concourse.mybir as mybir
import concourse.tile as tile
from concourse import bacc
from concourse.bass_utils import run_bass_kernel_spmd

BF16 = ml_dtypes.bfloat16
F8E4 = ml_dtypes.float8_e4m3
N_CORES = 8
SUP = 512
P = 128
N_DEST = 8192
D_MEM = 128

_PROG_CACHE: dict[int, object] = {}


def _build_program(caps: tuple):
    """Build the SPMD Bass program for per-position block caps."""
    offs = [0]
    for ck in caps:
        offs.append(offs[-1] + ck)
    e_cap = offs[-1]           # padded edges per core

    nc = bacc.Bacc("TRN2", target_bir_lowering=False, debug=False,
                   num_devices=N_CORES)
    f32 = mybir.dt.float32
    bf16 = mybir.dt.bfloat16
    fp8 = mybir.dt.float8e4
    AF = mybir.ActivationFunctionType
    OP = mybir.AluOpType

    srcT = nc.dram_tensor("srcT", [2, P, e_cap], fp8, kind="ExternalInput")
    efts = nc.dram_tensor("efts", [97, e_cap], fp8, kind="ExternalInput")
    S_d = nc.dram_tensor("S_d", [P, e_cap], fp8, kind="ExternalInput")
    # constants coalesced into 3 transfers: 17 separate 128-descriptor const
    # loads flood the DGE ring and delay the first stream chunks
    cb16 = nc.dram_tensor("cb16", [P, 3200], bf16, kind="ExternalInput")
    cf32 = nc.dram_tensor("cf32", [P, 12], f32, kind="ExternalInput")
    wm1 = nc.dram_tensor("wm1", [97, P], fp8, kind="ExternalInput")
    out_d = nc.dram_tensor("writeT", [P, 1024], f32, kind="ExternalOutput")

    with tile.TileContext(nc) as tc:
        with (
            tc.tile_pool(name="const", bufs=1) as cp,
            tc.tile_pool(name="io", bufs=8) as iop,
            tc.tile_pool(name="mid", bufs=8) as midp,
            tc.tile_pool(name="rdps", bufs=2, space="PSUM") as rdps,
            tc.tile_pool(name="mgps", bufs=2, space="PSUM") as mgps,
            tc.tile_pool(name="aggps", bufs=2, space="PSUM") as aggps,
            tc.tile_pool(name="dstps", bufs=1, space="PSUM") as dstps,
            tc.tile_pool(name="tps", bufs=1, space="PSUM") as tps,
        ):
            cb = cp.tile([P, 3200], bf16, tag="cb16")
            nc.sync.dma_start(out=cb[:], in_=cb16[:, :])
            cf = cp.tile([P, 12], f32, tag="cf32")
            nc.scalar.dma_start(out=cf[:], in_=cf32[:, :])
            wm1_t = cp.tile([97, P], fp8, tag="wm1")
            nc.scalar.dma_start(out=wm1_t[:], in_=wm1[:, :])

            dstT0 = cb[:, 0:1024]
            dstT1 = cb[:, 1024:2048]
            wr0 = cb[:, 2048:2176]
            wr1 = cb[:, 2176:2304]
            wm0_t = cb[:, 2304:2432]
            wa0 = cb[:, 2432:2560]
            wa1 = cb[:, 2560:2688]
            wu0 = cb[:, 2688:2816]
            wu1 = cb[:, 2816:2944]
            ww_t = cb[:, 2944:3072]
            ident_t = cb[:, 3072:3200]
            br_t = cf[:, 0:1]
            ba_t = cf[:, 1:2]
            bu_t = cf[:, 2:3]
            bw_t = cf[:, 3:4]
            inv_t = cf[:, 4:12]

            def dst_stage(b, agg_ps, stage, hold):
                dc = slice(b * P, (b + 1) * P)
                if stage == 0:
                    # agg_ps is [d, m] msg_sum; scale rows by 1/cnt on ACT,
                    # transpose on PE back to [m, d] for the dst-side chain
                    mm_dm = midp.tile([P, P], bf16, tag="mmdm")
                    nc.scalar.mul(mm_dm[:], agg_ps[:], inv_t[:, b:b + 1])
                    tp = tps.tile([P, P], bf16, tag="tp")
                    nc.tensor.transpose(tp[:], mm_dm[:], ident_t[:])
                    mmean = midp.tile([P, P], bf16, tag="mmean")
                    nc.vector.tensor_copy(mmean[:], tp[:])
                    drp = dstps.tile([P, P], f32, tag="dst")
                    nc.tensor.matmul(drp[:], lhsT=wr0[:], rhs=dstT0[:, dc],
                                     start=True, stop=False)
                    nc.tensor.matmul(drp[:], lhsT=wr1[:], rhs=dstT1[:, dc],
                                     start=False, stop=True)
                    dstr = midp.tile([P, P], bf16, tag="dstr")
                    nc.scalar.activation(dstr[:], drp[:], AF.Relu, bias=br_t[:, :1])
                    hold.update(mmean=mmean, dstr=dstr)
                elif stage == 1:
                    agp = dstps.tile([P, P], f32, tag="dst")
                    nc.tensor.matmul(agp[:], lhsT=wa0[:], rhs=hold["dstr"][:],
                                     start=True, stop=False)
                    nc.tensor.matmul(agp[:], lhsT=wa1[:], rhs=hold["mmean"][:],
                                     start=False, stop=True)
                    aggT = midp.tile([P, P], bf16, tag="aggT")
                    nc.scalar.activation(aggT[:], agp[:], AF.Relu, bias=ba_t[:, :1])
                    hold.update(aggT=aggT)
                elif stage == 2:
                    upp = dstps.tile([P, P], f32, tag="dst")
                    nc.tensor.matmul(upp[:], lhsT=wu0[:], rhs=hold["aggT"][:],
                                     start=True, stop=False)
                    nc.tensor.matmul(upp[:], lhsT=wu1[:], rhs=hold["dstr"][:],
                                     start=False, stop=True)
                    updT = midp.tile([P, P], bf16, tag="updT")
                    nc.scalar.activation(updT[:], upp[:], AF.Relu, bias=bu_t[:, :1])
                    hold.update(updT=updT)
                else:
                    wrp = dstps.tile([P, P], f32, tag="dst")
                    nc.tensor.matmul(wrp[:], lhsT=ww_t[:], rhs=hold["updT"][:],
                                     start=True, stop=True)
                    wout = midp.tile([P, P], f32, tag="wout")
                    nc.scalar.activation(wout[:], wrp[:], AF.Tanh, bias=bw_t[:, :1])
                    nc.sync.dma_start(out=out_d[:, dc], in_=wout[:])

            pending = None
            hold = {}
            for b in range(8):
                bc = caps[b]
                widths = [SUP] * (bc // SUP)
                if bc % SUP:
                    widths.append(bc % SUP)
                nst = len(widths)
                agg_ps = aggps.tile([P, P], f32, tag="agg")
                dbl = {}
                for st, w in enumerate(widths):
                    off = offs[b] + st * SUP
                    if st % 2 == 0:
                        wch = min(2 * SUP, bc - st * SUP)
                        s0d = iop.tile([P, 2 * SUP], fp8, tag="s0")
                        nc.sync.dma_start(out=s0d[:, :wch],
                                          in_=srcT[0, :, off:off + wch])
                        s1d = iop.tile([P, 2 * SUP], fp8, tag="s1")
                        nc.scalar.dma_start(out=s1d[:, :wch],
                                            in_=srcT[1, :, off:off + wch])
                        efd = iop.tile([97, 2 * SUP], fp8, tag="ef")
                        nc.gpsimd.dma_start(out=efd[:, :wch],
                                            in_=efts[:, off:off + wch])
                        S4d = iop.tile([P, 2 * SUP], fp8, tag="S4")
                        nc.sync.dma_start(out=S4d[:, :wch],
                                          in_=S_d[:, off:off + wch])
                        dbl = dict(s0d=s0d, s1d=s1d, efd=efd, S4d=S4d)
                    half = (st % 2) * SUP
                    hs = slice(half, half + w)
                    s0, s1 = dbl["s0d"][:, hs], dbl["s1d"][:, hs]
                    ef, S4 = dbl["efd"][:, hs], dbl["S4d"][:, hs]
                    nq = w // P

                    rd = rdps.tile([P, SUP], f32, tag="rd")
                    nc.tensor.matmul(rd[:, :w], lhsT=wr0[:], rhs=s0,
                                     start=True, stop=False)
                    nc.tensor.matmul(rd[:, :w], lhsT=wr1[:], rhs=s1,
                                     start=False, stop=True)
                    srT = midp.tile([P, SUP], bf16, tag="srT")
                    nc.scalar.activation(srT[:, :w], rd[:, :w], AF.Relu,
                                         bias=br_t[:, :1])

                    mg4 = mgps.tile([P, SUP], f32, tag="mg")
                    for q in range(nq):
                        qs = slice(q * P, (q + 1) * P)
                        nc.tensor.matmul(mg4[:, qs], lhsT=srT[:, qs],
                                         rhs=wm0_t[:], start=True, stop=False)
                        nc.tensor.matmul(mg4[:, qs], lhsT=ef[:, qs],
                                         rhs=wm1_t[:], start=False, stop=True)
                    msgs4 = midp.tile([P, SUP], fp8, tag="msgs")
                    nc.vector.tensor_scalar_max(msgs4[:, :w], mg4[:, :w], 0.0)
                    # fp8 DoubleRow: two 128-edge subtiles per matmul
                    q = 0
                    while q < nq:
                        if q + 1 < nq:
                            qs = slice(q * P, (q + 2) * P)
                            S2 = S4[:, qs].rearrange(
                                "p (two n) -> p two n", two=2)
                            M2 = msgs4[:, qs].rearrange(
                                "p (two n) -> p two n", two=2)
                            nc.tensor.matmul(
                                agg_ps[:], lhsT=S2, rhs=M2,
                                start=(st == 0 and q == 0),
                                stop=(st == nst - 1 and q + 2 >= nq),
                                perf_mode=mybir.MatmulPerfMode.DoubleRow)
                            q += 2
                        else:
                            qs = slice(q * P, (q + 1) * P)
                            nc.tensor.matmul(
                                agg_ps[:], lhsT=S4[:, qs], rhs=msgs4[:, qs],
                                start=(st == 0 and q == 0),
                                stop=(st == nst - 1 and q + 1 >= nq))
                            q += 1

                    if pending is not None and st < 4:
                        dst_stage(pending[0], pending[1], st, hold)
                        if st == 3:
                            pending = None
                if pending is not None:
                    for stage in range(min(nst, 4), 4):
                        dst_stage(pending[0], pending[1], stage, hold)
                pending = (b, agg_ps)
                hold = {}
            for stage in range(4):
                dst_stage(pending[0], pending[1], stage, hold)

    nc.finalize()
    return nc


def _prep_inputs(inputs):
    """Host-side shard/pack. Returns (in_maps, caps, node_memory, scatter_ids)."""
    node_memory = np.ascontiguousarray(np.asarray(inputs["node_memory"], np.float32))
    node_features = np.asarray(inputs["node_features"], np.float32)
    edge_features = np.asarray(inputs["edge_features"], np.float32)
    time_encoding = np.asarray(inputs["time_encoding"], np.float32)
    node_ids = np.asarray(inputs["node_ids"]).astype(np.int64)
    source_ids = np.asarray(inputs["source_ids"]).astype(np.int64)
    edge_ids = np.asarray(inputs["edge_ids"]).astype(np.int64)
    dest_seg = np.asarray(inputs["dest_seg"]).astype(np.int64)
    W_read = np.asarray(inputs["W_read"], np.float32)
    b_read = np.asarray(inputs["b_read"], np.float32)
    W_msg = np.asarray(inputs["W_msg"], np.float32)
    b_msg = np.asarray(inputs["b_msg"], np.float32)
    W_agg = np.asarray(inputs["W_agg"], np.float32)
    b_agg = np.asarray(inputs["b_agg"], np.float32)
    W_upd = np.asarray(inputs["W_upd"], np.float32)
    b_upd = np.asarray(inputs["b_upd"], np.float32)
    W_write = np.asarray(inputs["W_write"], np.float32)
    b_write = np.asarray(inputs["b_write"], np.float32)

    n_edge = dest_seg.shape[0]

    cnt = np.bincount(dest_seg, minlength=N_DEST)
    inv_cnt = np.zeros(N_DEST, np.float32)
    nz = cnt > 0
    inv_cnt[nz] = 1.0 / cnt[nz]

    # 64 global dest blocks of 128 contiguous dest segments, sorted by edge
    # count and dealt round-robin: position k on core c gets block
    # order[k*8+c], so per-position caps (max over cores, rounded to 128)
    # waste ~9% less padding than a uniform cap and keep SPMD symmetric.
    bounds = np.searchsorted(dest_seg, np.arange(0, N_DEST + 1, P))
    per_block = np.diff(bounds)
    order = np.argsort(-per_block, kind="stable")
    blk_ids = order.reshape(8, N_CORES)          # [position, core]
    caps = tuple(int(math.ceil(per_block[blk_ids[k]].max() / P)) * P
                 for k in range(8))
    offs = np.concatenate([[0], np.cumsum(caps)]).astype(np.int64)
    e_cap = int(offs[-1])
    nt = e_cap // P

    # per-core edge selection (padded); esel indexes into the edge arrays
    esel = np.zeros((N_CORES, e_cap), np.int64)
    valid = np.zeros((N_CORES, e_cap), bool)
    for c in range(N_CORES):
        for k in range(8):
            B = int(blk_ids[k, c])
            lo, hi = int(bounds[B]), int(bounds[B + 1])
            off = int(offs[k])
            esel[c, off:off + hi - lo] = np.arange(lo, hi)
            valid[c, off:off + hi - lo] = True
    esel_f = esel.reshape(-1)
    valid_f = valid.reshape(-1)

    # per-core dest segment ids in position order, [c, 1024]
    dseg_ids = np.stack([
        np.concatenate([np.arange(int(blk_ids[k, c]) * P,
                                  int(blk_ids[k, c]) * P + P)
                        for k in range(8)])
        for c in range(N_CORES)])
    scatter_ids = node_ids[dseg_ids]             # [c, 1024] rows to write

    nodecat = np.concatenate([node_memory, node_features], axis=1)  # [N,256]

    src_rows = nodecat[source_ids[esel_f]]
    src_rows[~valid_f] = 0.0
    srcT = np.ascontiguousarray(
        src_rows.reshape(N_CORES, e_cap, 256).transpose(0, 2, 1)
    ).astype(F8E4).reshape(N_CORES, 2, P, e_cap)

    ef_rows = edge_features[edge_ids[esel_f]]
    t_rows = time_encoding[np.minimum(esel_f, n_edge - 1)]
    eft = np.concatenate(
        [ef_rows, t_rows, np.ones((len(esel_f), 1), np.float32)], axis=1)
    eft[~valid_f] = 0.0
    eft[valid_f, 96] = 1.0
    efts = np.ascontiguousarray(
        eft.reshape(N_CORES, e_cap, 97).transpose(0, 2, 1)).astype(F8E4)

    # unscaled one-hot S (0/1, exact in fp8); 1/cnt applied post-aggregation
    ld_e = dest_seg[esel_f] % P
    ld_e[~valid_f] = 0
    S_flat = np.zeros((N_CORES * e_cap, P), np.float32)
    S_flat[np.arange(N_CORES * e_cap), ld_e] = valid_f.astype(np.float32)
    S_pack = np.ascontiguousarray(
        S_flat.reshape(N_CORES, nt, P, P).transpose(0, 2, 1, 3)
        .reshape(N_CORES, P, e_cap)).astype(F8E4)

    # per-dest 1/cnt, [c, 128, 8]: column b = block b's local dests
    inv_pack = np.ascontiguousarray(
        inv_cnt[dseg_ids].reshape(N_CORES, 8, P).transpose(0, 2, 1)
    ).astype(np.float32)

    drows = nodecat[scatter_ids.reshape(-1)]     # [8192, 256] position order
    dstT = np.ascontiguousarray(
        drows.reshape(N_CORES, 1024, 256).transpose(0, 2, 1)
    ).astype(BF16).reshape(N_CORES, 2, P, 1024)

    wr_h = W_read.reshape(2, P, P).astype(BF16)
    wm1_h = np.ascontiguousarray(
        np.concatenate([W_msg[P:], b_msg[None, :]], axis=0)).astype(F8E4)
    wa_h = W_agg.reshape(2, P, P).astype(BF16)
    wu_h = W_upd.reshape(2, P, P).astype(BF16)

    # bf16 const blob [c, 128, 3200]
    cb16 = np.zeros((N_CORES, P, 3200), BF16)
    cb16[:, :, 0:1024] = dstT[:, 0]
    cb16[:, :, 1024:2048] = dstT[:, 1]
    for c in range(N_CORES):
        cb16[c, :, 2048:2176] = wr_h[0]
        cb16[c, :, 2176:2304] = wr_h[1]
        cb16[c, :, 2304:2432] = W_msg[:P].astype(BF16)
        cb16[c, :, 2432:2560] = wa_h[0]
        cb16[c, :, 2560:2688] = wa_h[1]
        cb16[c, :, 2688:2816] = wu_h[0]
        cb16[c, :, 2816:2944] = wu_h[1]
        cb16[c, :, 2944:3072] = W_write.astype(BF16)
        cb16[c, :, 3072:3200] = np.eye(P, dtype=BF16)
    cb16 = np.ascontiguousarray(cb16)

    # f32 const blob [c, 128, 12]: br | ba | bu | bw | inv[8]
    cf32 = np.zeros((N_CORES, P, 12), np.float32)
    cf32[:, :, 0] = b_read[None, :]
    cf32[:, :, 1] = b_agg[None, :]
    cf32[:, :, 2] = b_upd[None, :]
    cf32[:, :, 3] = b_write[None, :]
    cf32[:, :, 4:12] = inv_pack
    cf32 = np.ascontiguousarray(cf32)

    in_maps = []
    for c in range(N_CORES):
        in_maps.append({
            "srcT": srcT[c], "efts": efts[c], "S_d": S_pack[c],
            "cb16": cb16[c], "cf32": cf32[c], "wm1": wm1_h,
        })
    return in_maps, caps, node_memory, scatter_ids


def run(inputs, trace=False, **kw):
    in_maps, caps, node_memory, scatter_ids = _prep_inputs(inputs)
    if caps not in _PROG_CACHE:
        _PROG_CACHE[caps] = _build_program(caps)
    nc = _PROG_CACHE[caps]
    res = run_bass_kernel_spmd(nc, in_maps, core_ids=list(range(N_CORES)),
                               trace=trace, **kw)
    out = node_memory.copy()
    for c in range(N_CORES):
        wt = np.asarray(res.results[c]["writeT"], np.float32).T  # [1024,128]
        out[scatter_ids[c]] = wt
    return out, res


def kernel(**inputs) -> np.ndarray:
    out, _ = run(inputs, trace=False)
    return out

